# revision 37
# baseline (speedup 1.0000x reference)
"""EpisodicMemory kernel for Trainium2, 8-core data-parallel.

Reference computation (per batch b, d=32, m=64 memory slots, 2 hops):
    M = vs[b]
    for hop:
        Rh[m,:] = R[b,hop,m] @ h[b,hop,m]                  # batched matvec
        z = [Rh*v, Rh*M, |Rh-v|, |Rh-M|]                   # [m, 4d]
        Z = tanh(z @ W1.T + b1) @ W2.T (+ b2: dropped — softmax-invariant)
        g = softmax(Z over m); o = sum_m ts[b,hop,m] * g[m]
        M = GRUCell(o, M)
    out[b] = M

Sharding: pure data parallel over batch; 128 batches per core.

Numerics: Rs/hs/ts are host-cast to bf16 (DMA bytes halve; DVE runs 2-byte
packed ops at 2x). Einsum reduce over e is an in-place bf16 add-tree on DVE
(TensorReduce gets no 2x mode; the tree does). Softmax and the GRU stay f32.

Per-core layout strategy:
  - einsum Rh: R tiles [128 part=(m,bp), free=(g4,d32,e32)] (b = blk*8+bp*4+g;
    p = m*2+bp, so one 128-partition DMA per block covers 1 MiB at full rate),
    mul (in-place, h broadcast over d) on DVE/Pool alternating, then a 5-level
    in-place add-tree over e on DVE.
  - features built in row layout [128 rows, (g,f,d)] bf16, PE-transposed to
    z^T [feat128, rows] for the MLP matmuls (bf16) on TensorE.
  - softmax/o batched per hop over all 128 batches [128 part=b, 64 m] in f32.
  - GRU in transposed layout [d part, b free], f32; M kept as MT [32,128],
    M_rep rebuilt in bf16 via a DRAM broadcast bounce.
"""

import numpy as np

import concourse.bacc as bacc
import concourse.bass as bass
import concourse.mybir as mybir
import concourse.tile as tile
from concourse.masks import make_identity
from concourse.tile import add_dep_helper

F32 = mybir.dt.float32
BF16 = mybir.dt.bfloat16
AF = mybir.ActivationFunctionType
ALU = mybir.AluOpType
AX = mybir.AxisListType

B, N_HOP, N_MEM, DIM = 1024, 2, 64, 32
N_CORES = 8
BC = B // N_CORES            # 128 batches per core
BB = 8                       # batches per block
NBLK = BC // BB              # 16 blocks
NG = BB // 2                 # 4 b-pair groups per block
ROWS = BB * N_MEM            # 512 rows per block
D4 = 4 * DIM                 # 128 MLP input features


def build_nc(n_iter: int = 1, variant: str = "full") -> bass.Bass:
    """variant: 'full' | 'dma' (loads only) | 'nodma' (R loaded once) |
    'nopool' (all muls on DVE) | 'allpool' (all muls on Pool)."""
    nc = bacc.Bacc("TRN2")

    # Rs/hs arrive host-permuted: [hop, blk, m, bp, g, ...] with b = blk*8+bp*4+g
    Rs_d = nc.dram_tensor(
        "Rs", [N_HOP, NBLK, N_MEM, 2, NG, DIM, DIM], BF16, kind="ExternalInput"
    )
    hs_d = nc.dram_tensor(
        "hs", [N_HOP, N_MEM, 2, NBLK, NG, DIM], BF16, kind="ExternalInput"
    )
    ts_d = nc.dram_tensor("ts", [BC, N_HOP, N_MEM, DIM], BF16, kind="ExternalInput")
    vs_d = nc.dram_tensor("vs", [BC, DIM], F32, kind="ExternalInput")
    W1_d = nc.dram_tensor("W1", [DIM, D4], F32, kind="ExternalInput")
    b1_d = nc.dram_tensor("b1", [DIM], F32, kind="ExternalInput")
    W2_d = nc.dram_tensor("W2", [1, DIM], F32, kind="ExternalInput")
    Wih_d = nc.dram_tensor("W_ih", [N_HOP, 3 * DIM, DIM], F32, kind="ExternalInput")
    Whh_d = nc.dram_tensor("W_hh", [N_HOP, 3 * DIM, DIM], F32, kind="ExternalInput")
    bih_d = nc.dram_tensor("b_ih", [N_HOP, 3 * DIM], F32, kind="ExternalInput")
    bhh_d = nc.dram_tensor("b_hh", [N_HOP, 3 * DIM], F32, kind="ExternalInput")
    out_d = nc.dram_tensor("out", [BC, DIM], F32, kind="ExternalOutput")
    # DRAM bounces for the v/M partition-broadcast (bf16)
    m_flat = nc.dram_tensor("m_flat", [BC, DIM], BF16)
    m_scr2 = nc.dram_tensor("m_scratch2", [2, NBLK, NG, DIM], BF16)
    m_scr3 = nc.dram_tensor("m_scratch3", [128, NBLK * NG * DIM], BF16)
    v_flat = nc.dram_tensor("v_flat", [BC, DIM], BF16)
    v_scr = nc.dram_tensor("v_scratch", [2, NBLK, NG, DIM], BF16)
    v_scr3 = nc.dram_tensor("v_scratch3", [128, NBLK * NG * DIM], BF16)
    # Z bounce, laid out so the gather is contiguous per natural batch index
    z_scr = nc.dram_tensor("z_scratch", [NBLK, 2, NG, N_MEM], F32)

    import contextlib

    with tile.TileContext(nc) as tc:
        with (
            (tc.For_i(0, n_iter, 1) if n_iter > 1 else contextlib.nullcontext()),
            tc.tile_pool(name="consts", bufs=1) as consts,
            tc.tile_pool(name="hop_io", bufs=2) as hop_io,
            tc.tile_pool(name="rpool", bufs=6) as rpool,
            tc.tile_pool(name="fpool", bufs=4) as fpool,
            tc.tile_pool(name="zpool", bufs=4) as zpool,
            tc.tile_pool(name="apool", bufs=4) as apool,
            tc.tile_pool(name="small", bufs=2) as small,
            tc.tile_pool(name="mstate", bufs=2) as mstate,
            tc.tile_pool(name="pp_z", bufs=3, space="PSUM") as pp_z,
            tc.tile_pool(name="pp_1", bufs=2, space="PSUM") as pp_1,
            tc.tile_pool(name="pp_2", bufs=1, space="PSUM") as pp_2,
            tc.tile_pool(name="pp_g", bufs=2, space="PSUM") as pp_g,
        ):
            ident = consts.tile([128, 128], F32)
            make_identity(nc, ident)
            ident16 = consts.tile([128, 128], BF16)
            nc.scalar.copy(out=ident16, in_=ident)

            # ---- weights prep (one-time) ----
            w1_sb = consts.tile([DIM, D4], F32)
            nc.sync.dma_start(out=w1_sb, in_=W1_d[:, :])
            w1t_ps = pp_g.tile([D4, DIM], F32, tag="gpsum")
            nc.tensor.transpose(w1t_ps, w1_sb, ident[:DIM, :DIM])
            W1T = consts.tile([D4, DIM], BF16)
            nc.scalar.copy(out=W1T, in_=w1t_ps)

            W2T_f = consts.tile([DIM, 1], F32)
            nc.sync.dma_start(out=W2T_f, in_=W2_d.rearrange("a b -> b a"))
            W2T = consts.tile([DIM, 1], BF16)
            nc.scalar.copy(out=W2T, in_=W2T_f)
            b1T = consts.tile([DIM, 1], F32)
            nc.sync.dma_start(out=b1T, in_=b1_d[:].unsqueeze(1))

            WihT, WhhT, bsum_rz, bihn_t, bhhn_t = [], [], [], [], []
            for hop in range(N_HOP):
                wih_sb = consts.tile([3 * DIM, DIM], F32, tag="wload", bufs=4)
                nc.sync.dma_start(out=wih_sb, in_=Wih_d[hop])
                wt_ps = pp_g.tile([DIM, 3 * DIM], F32, tag="gpsum")
                nc.tensor.transpose(wt_ps, wih_sb, ident[: 3 * DIM, : 3 * DIM])
                wT = consts.tile([DIM, 3 * DIM], F32, tag=f"wihT{hop}")
                nc.scalar.copy(out=wT, in_=wt_ps)
                WihT.append(wT)

                whh_sb = consts.tile([3 * DIM, DIM], F32, tag="wload", bufs=4)
                nc.sync.dma_start(out=whh_sb, in_=Whh_d[hop])
                wt_ps2 = pp_g.tile([DIM, 3 * DIM], F32, tag="gpsum")
                nc.tensor.transpose(wt_ps2, whh_sb, ident[: 3 * DIM, : 3 * DIM])
                wT2 = consts.tile([DIM, 3 * DIM], F32, tag=f"whhT{hop}")
                nc.scalar.copy(out=wT2, in_=wt_ps2)
                WhhT.append(wT2)

                # per-gate bias tiles, all at base partition 0
                gate_b = []
                for gd, gname in ((bih_d, "ih"), (bhh_d, "hh")):
                    for gate in range(3):
                        bt = consts.tile([DIM, 1], F32, tag=f"b{gname}{hop}{gate}")
                        nc.sync.dma_start(
                            out=bt,
                            in_=gd[hop, gate * DIM : (gate + 1) * DIM].unsqueeze(1),
                        )
                        gate_b.append(bt)
                b_r = consts.tile([DIM, 1], F32, tag=f"b_r{hop}")
                nc.vector.tensor_add(b_r, gate_b[0], gate_b[3])
                b_z = consts.tile([DIM, 1], F32, tag=f"b_z{hop}")
                nc.vector.tensor_add(b_z, gate_b[1], gate_b[4])
                bsum_rz.append((b_r, b_z))
                bihn_t.append(gate_b[2])
                bhhn_t.append(gate_b[5])

            # ---- initial M state ----
            vs_row = consts.tile([BC, DIM], F32)
            nc.sync.dma_start(out=vs_row, in_=vs_d[:, :])
            vst_ps = pp_g.tile([DIM, BC], F32, tag="gpsum")
            nc.tensor.transpose(vst_ps, vs_row, ident)
            vsT = consts.tile([DIM, BC], F32)
            nc.scalar.copy(out=vsT, in_=vst_ps)
            MT = vsT  # current M^T [d, b]

            # v_rep [128 part=(m,bp), (blk,g,d)] bf16: value vs[blk*8+bp*4+g, d].
            # Cast to bf16, stage a (bp, blk, g, d)-permuted copy in DRAM, then
            # replicate per-partition rows in DRAM (free-form APs), then a
            # plain [128, f] load (SBUF DMA APs must be partition-clean on HW).
            vs16 = consts.tile([BC, DIM], BF16)
            nc.scalar.copy(out=vs16, in_=vs_row)
            nc.sync.dma_start(out=v_flat[:, :], in_=vs16)
            nc.sync.dma_start(
                out=v_scr[:, :, :, :],
                in_=v_flat.rearrange("(blk bp g) d -> bp blk g d", bp=2, g=NG),
            )
            nc.sync.dma_start(
                out=v_scr3.rearrange("(m bp) f -> m bp f", bp=2),
                in_=v_scr.rearrange(
                    "bp blk g d -> bp (blk g d)"
                ).partition_broadcast(64),
            )
            v_rep = consts.tile([128, NBLK * NG * DIM], BF16)
            prev_bcast_dma = nc.sync.dma_start(out=v_rep, in_=v_scr3[:, :])

            M_rep = v_rep  # hop 0: M == vs

            for hop in range(N_HOP):
                # per-hop h in einsum layout [(bp,m), (blk,g,e)]
                # h for the whole hop: one contiguous [128, 2048] bf16 load
                h_hop = hop_io.tile([128, NBLK * NG * DIM], BF16, tag="h_hop")
                nc.scalar.dma_start(
                    out=h_hop,
                    in_=hs_d[hop].rearrange("m bp blk g e -> (m bp) (blk g e)"),
                )
                # t natural layout [b, (m,d)]
                t_hop = hop_io.tile([BC, N_MEM * DIM], BF16, tag="t_hop")
                t_dma = nc.scalar.dma_start(
                    out=t_hop, in_=ts_d[:, hop].rearrange("b m d -> b (m d)")
                )
                # lane-ordering: keep the broadcast DMA strictly before t_hop
                add_dep_helper(t_dma.ins, prev_bcast_dma.ins,
                               reason="hwdge lane ordering")

                Z_row = small.tile([BC, N_MEM], F32, tag="Z_row")

                for blk in range(NBLK):
                    if variant == "nodma":
                        if hop == 0 and blk == 0:
                            r_tile = consts.tile(
                                [128, NG * DIM * DIM], BF16, tag="Rconst"
                            )
                            nc.sync.dma_start(
                                out=r_tile,
                                in_=Rs_d[hop, blk].rearrange(
                                    "m bp g d e -> (m bp) (g d e)"
                                ),
                            )
                            r_const = r_tile
                        r_tile = rpool.tile([128, NG * DIM * DIM], BF16, tag="R")
                        nc.vector.tensor_copy(r_tile, r_const)
                    else:
                        r_tile = rpool.tile([128, NG * DIM * DIM], BF16, tag="R")
                        nc.sync.dma_start(
                            out=r_tile,
                            in_=Rs_d[hop, blk].rearrange(
                                "m bp g d e -> (m bp) (g d e)"
                            ),
                        )
                    if variant == "dma":
                        continue
                    # P = R * h (in-place), h broadcast over d
                    r4 = r_tile.rearrange("p (g d e) -> p g d e", g=NG, d=DIM)
                    h_slice = h_hop[
                        :, blk * NG * DIM : (blk + 1) * NG * DIM
                    ].rearrange("p (g e) -> p g e", g=NG)
                    # odd blocks: broadcast-mul on Pool/GPSIMD (engine-time
                    # indifferent to the stride-0 operand) so it overlaps DVE.
                    on_pool = blk % 2 == 1
                    if variant == "nopool":
                        on_pool = False
                    elif variant == "allpool":
                        on_pool = True
                    if on_pool:
                        h_v = h_slice.unsqueeze(2).broadcast_to(
                            (128, NG, DIM, DIM)
                        )
                        nc.gpsimd.tensor_tensor(r4, r4, h_v, op=ALU.mult)
                    else:
                        # DVE: the stride-0 broadcast operand defeats the DVE
                        # 2-byte fast path on HW, so materialize h replicated
                        # over d via log-doubling packed copies, then a fully
                        # packed mul.
                        hr = fpool.tile([128, NG * DIM * DIM], BF16, tag="hrep")
                        h4 = hr.rearrange("p (g d e) -> p g d e", g=NG, d=DIM)
                        nc.vector.tensor_copy(
                            h4[:, :, 0:1, :], h_slice.unsqueeze(2)
                        )
                        for dlo, dhi in ((1, 2), (2, 4), (4, 8), (8, 16), (16, 32)):
                            nc.vector.tensor_copy(
                                h4[:, :, dlo:dhi, :], h4[:, :, 0 : dhi - dlo, :]
                            )
                        nc.vector.tensor_mul(r_tile, r_tile, hr)
                    # Rh[(bp,m), (g,d)] = sum_e P via in-place bf16 add-tree
                    # (TensorReduce gets no 2-byte 2x mode; packed adds do)
                    p3 = r_tile.rearrange("p (gd e) -> p gd e", e=DIM)
                    for half in (16, 8, 4, 2):
                        nc.vector.tensor_add(
                            p3[:, :, :half], p3[:, :, :half], p3[:, :, half : 2 * half]
                        )
                    rh = fpool.tile([128, NG * DIM], BF16, tag="rh")
                    nc.vector.tensor_add(
                        rh.rearrange("p (gd o) -> p gd o", o=1),
                        p3[:, :, 0:1],
                        p3[:, :, 1:2],
                    )
                    # features F [(bp,m), (g, f, d)] bf16
                    f_blk = fpool.tile([128, NG * 4 * DIM], BF16, tag="F")
                    f4 = f_blk.rearrange("p (g f d) -> p g f d", g=NG, f=4)
                    rh3 = rh.rearrange("p (g d) -> p g d", g=NG)
                    vr3 = v_rep[:, blk * NG * DIM : (blk + 1) * NG * DIM].rearrange(
                        "p (g d) -> p g d", g=NG
                    )
                    mr3 = M_rep[:, blk * NG * DIM : (blk + 1) * NG * DIM].rearrange(
                        "p (g d) -> p g d", g=NG
                    )
                    nc.vector.tensor_mul(f4[:, :, 0, :], rh3, vr3)
                    nc.vector.tensor_mul(f4[:, :, 1, :], rh3, mr3)
                    nc.vector.tensor_sub(f4[:, :, 2, :], rh3, vr3)
                    nc.vector.tensor_sub(f4[:, :, 3, :], rh3, mr3)
                    nc.scalar.activation(f4[:, :, 2, :], f4[:, :, 2, :], AF.Abs)
                    nc.scalar.activation(f4[:, :, 3, :], f4[:, :, 3, :], AF.Abs)

                    # transpose to z^T [(f,d), (g,bp,m)]
                    zt_ps = pp_z.tile([D4, ROWS], BF16, tag="zt")
                    for g in range(NG):
                        nc.tensor.transpose(
                            zt_ps[:, g * 128 : (g + 1) * 128],
                            f_blk[:, g * 128 : (g + 1) * 128],
                            ident16,
                        )
                    # PSUM->SBUF copies on DVE (idle during the MLP burst;
                    # Act is the pacer there)
                    zt_sb = zpool.tile([D4, ROWS], BF16, tag="zt_sb")
                    nc.vector.tensor_copy(zt_sb, zt_ps)

                    ps1 = pp_1.tile([DIM, ROWS], F32, tag="ps1")
                    nc.tensor.matmul(ps1, lhsT=W1T, rhs=zt_sb, start=True, stop=True)
                    a1 = apool.tile([DIM, ROWS], BF16, tag="a1")
                    nc.scalar.activation(a1, ps1, AF.Tanh, bias=b1T)
                    ps2 = pp_2.tile([1, ROWS], F32, tag="ps2")
                    nc.tensor.matmul(ps2, lhsT=W2T, rhs=a1, start=True, stop=True)
                    z_sb = zpool.tile([1, ROWS], F32, tag="z_sb")
                    nc.vector.tensor_copy(z_sb, ps2)
                    # z_sb free order is (g, m, bp); store as (bp, g, m).
                    # src stays 1-partition (dim0 count 1): split by bp.
                    # On the SP queue: Act's sequencer is busy enough.
                    for bp in range(2):
                        nc.sync.dma_start(
                            out=z_scr[blk, bp].unsqueeze(0),
                            in_=z_sb.rearrange("o (g m bp) -> o g m bp", g=NG, bp=2)[
                                :, :, :, bp
                            ],
                        )

                # gather Z rows from DRAM: flat (blk,bp,g) == natural b
                nc.sync.dma_start(
                    out=Z_row,
                    in_=z_scr.rearrange("a b c m -> (a b c) m"),
                )

                # softmax over m, batched over all 128 b. |Z| is small
                # (tanh-bounded second layer), so skip the max-subtract and
                # normalize o AFTER the t-reduction: o = (sum_m t*e) / sum_m e
                e_row = small.tile([BC, N_MEM], F32, tag="e_row")
                nc.scalar.activation(e_row, Z_row, AF.Exp)
                e16 = small.tile([BC, N_MEM], BF16, tag="e16")
                nc.scalar.copy(out=e16, in_=e_row)
                ssum = small.tile([BC, 1], F32, tag="ssum")
                nc.vector.tensor_reduce(out=ssum, in_=e_row, axis=AX.X, op=ALU.add)
                rsum = small.tile([BC, 1], F32, tag="rsum")
                nc.vector.reciprocal(rsum, ssum)

                # o[b,d] = (sum_m t[b,m,d] * e[b,m]) * rsum[b].
                # Expand e over d via packed doubling (broadcast APs defeat
                # the DVE 2-byte fast path on HW), packed mul, then a packed
                # add-tree over m.
                e_rep = small.tile([BC, N_MEM * DIM], BF16, tag="e_rep")
                er3 = e_rep.rearrange("b (m d) -> b m d", d=DIM)
                nc.vector.tensor_copy(er3[:, :, 0:1], e16.unsqueeze(2))
                for dlo, dhi in ((1, 2), (2, 4), (4, 8), (8, 16), (16, 32)):
                    nc.vector.tensor_copy(
                        er3[:, :, dlo:dhi], er3[:, :, 0 : dhi - dlo]
                    )
                t3 = t_hop.rearrange("b (m d) -> b m d", d=DIM)
                nc.vector.tensor_mul(t_hop, t_hop, e_rep)
                for mh in (32, 16, 8, 4, 2):
                    nc.vector.tensor_add(
                        t3[:, :mh, :], t3[:, :mh, :], t3[:, mh : 2 * mh, :]
                    )
                o_raw = small.tile([BC, DIM], F32, tag="o_raw")
                nc.vector.tensor_add(
                    o_raw.unsqueeze(1), t3[:, 0:1, :], t3[:, 1:2, :]
                )
                o_row = small.tile([BC, DIM], F32, tag="o_row")
                nc.vector.tensor_scalar_mul(o_row, o_raw, rsum)

                # GRU (transposed layout [*, b], f32)
                ot_ps = pp_g.tile([DIM, BC], F32, tag="gpsum")
                nc.tensor.transpose(ot_ps, o_row, ident)
                oT = small.tile([DIM, BC], F32, tag="oT")
                nc.scalar.copy(out=oT, in_=ot_ps)

                # per-gate matmuls so every gate tile sits at base partition 0
                def gate_pair(g):
                    gi = pp_g.tile([DIM, BC], F32, tag="gpsum")
                    nc.tensor.matmul(
                        gi,
                        lhsT=WihT[hop][:, g * DIM : (g + 1) * DIM],
                        rhs=oT,
                        start=True,
                        stop=True,
                    )
                    gh = pp_g.tile([DIM, BC], F32, tag="gpsum")
                    nc.tensor.matmul(
                        gh,
                        lhsT=WhhT[hop][:, g * DIM : (g + 1) * DIM],
                        rhs=MT,
                        start=True,
                        stop=True,
                    )
                    return gi, gh

                # r,z gates: sigmoid(gi + gh + b_ih + b_hh)
                rz_t = []
                for g in range(2):
                    gi, gh = gate_pair(g)
                    gb = small.tile([DIM, BC], F32, tag=f"g{g}b")
                    nc.scalar.activation(gb, gi, AF.Identity, bias=bsum_rz[hop][g])
                    nc.vector.tensor_add(gb, gb, gh)
                    gt = small.tile([DIM, BC], F32, tag=f"gate{g}")
                    nc.scalar.activation(gt, gb, AF.Sigmoid)
                    rz_t.append(gt)
                r_t, z_t = rz_t

                # n = tanh(gi_n + b_ih_n + r * (gh_n + b_hh_n))
                gi_n, gh_n = gate_pair(2)
                ghn = small.tile([DIM, BC], F32, tag="ghn")
                nc.scalar.activation(ghn, gh_n, AF.Identity, bias=bhhn_t[hop])
                gin = small.tile([DIM, BC], F32, tag="gin")
                nc.scalar.activation(gin, gi_n, AF.Identity, bias=bihn_t[hop])
                n1 = small.tile([DIM, BC], F32, tag="n1")
                nc.vector.tensor_mul(n1, r_t, ghn)
                nc.vector.tensor_add(n1, n1, gin)
                n_t = small.tile([DIM, BC], F32, tag="n_t")
                nc.scalar.activation(n_t, n1, AF.Tanh)

                # M' = n + z * (M - n)
                MT_new = mstate.tile([DIM, BC], F32, tag="MT")
                nc.vector.tensor_sub(MT_new, MT, n_t)
                nc.vector.tensor_mul(MT_new, MT_new, z_t)
                nc.vector.tensor_add(MT_new, MT_new, n_t)
                MT = MT_new

                if hop < N_HOP - 1:
                    # rebuild M_rep (bf16) via DRAM bounce
                    mrow_ps = pp_g.tile([BC, DIM], F32, tag="gpsum")
                    nc.tensor.transpose(mrow_ps, MT, ident[:DIM, :DIM])
                    M16 = mstate.tile([BC, DIM], BF16, tag="M16")
                    nc.scalar.copy(out=M16, in_=mrow_ps)
                    nc.scalar.dma_start(out=m_flat[:, :], in_=M16)
                    nc.sync.dma_start(
                        out=m_scr2[:, :, :, :],
                        in_=m_flat.rearrange(
                            "(blk bp g) d -> bp blk g d", bp=2, g=NG
                        ),
                    )
                    M_rep_new = mstate.tile(
                        [128, NBLK * NG * DIM], BF16, tag="M_rep", bufs=1
                    )
                    nc.sync.dma_start(
                        out=m_scr3.rearrange("(m bp) f -> m bp f", bp=2),
                        in_=m_scr2.rearrange(
                            "bp blk g d -> bp (blk g d)"
                        ).partition_broadcast(64),
                    )
                    prev_bcast_dma = nc.sync.dma_start(
                        out=M_rep_new, in_=m_scr3[:, :]
                    )
                    M_rep = M_rep_new
                else:
                    # row-major M only needed for the final output
                    mrow_ps = pp_g.tile([BC, DIM], F32, tag="gpsum")
                    nc.tensor.transpose(mrow_ps, MT, ident[:DIM, :DIM])
                    M_row = mstate.tile([BC, DIM], F32, tag="M_row")
                    nc.scalar.copy(out=M_row, in_=mrow_ps)
                    nc.sync.dma_start(out=out_d[:, :], in_=M_row)

    nc.compile()
    return nc


_NC_CACHE = None


def _get_nc():
    global _NC_CACHE
    if _NC_CACHE is None:
        _NC_CACHE = build_nc()
    return _NC_CACHE


def _bf16(x):
    import ml_dtypes

    return np.asarray(x).astype(ml_dtypes.bfloat16)


def permute_local(x):
    """[BC, N_HOP, m, ...] -> [N_HOP, NBLK, m, 2, NG, ...] with b = blk*8+bp*4+g."""
    tail = x.shape[2:]
    y = x.reshape(NBLK, 2, NG, N_HOP, *tail)
    order = (3, 0, 4, 1, 2) + tuple(range(5, y.ndim))
    return np.ascontiguousarray(y.transpose(order))


def permute_h(x):
    """hs [BC, N_HOP, m, e] -> [N_HOP, m, 2, NBLK, NG, e]."""
    y = x.reshape(NBLK, 2, NG, N_HOP, N_MEM, DIM)
    return np.ascontiguousarray(y.transpose(3, 4, 1, 0, 2, 5))


def make_in_maps(hs, Rs, ts, vs, W1, b1, W2, W_ih, W_hh, b_ih, b_hh):
    in_maps = []
    for c in range(N_CORES):
        sl = slice(c * BC, (c + 1) * BC)
        in_maps.append(
            {
                "Rs": permute_local(_bf16(Rs[sl])),
                "hs": permute_h(_bf16(hs[sl])),
                "ts": _bf16(ts[sl]),
                "vs": np.ascontiguousarray(vs[sl]),
                "W1": np.ascontiguousarray(W1),
                "b1": np.ascontiguousarray(b1),
                "W2": np.ascontiguousarray(W2),
                "W_ih": np.ascontiguousarray(W_ih),
                "W_hh": np.ascontiguousarray(W_hh),
                "b_ih": np.ascontiguousarray(b_ih),
                "b_hh": np.ascontiguousarray(b_hh),
            }
        )
    return in_maps


def kernel(hs, Rs, ts, vs, W1, b1, W2, b2, W_ih, W_hh, b_ih, b_hh):
    from concourse.bass_utils import run_bass_kernel_spmd

    nc = _get_nc()
    in_maps = make_in_maps(hs, Rs, ts, vs, W1, b1, W2, W_ih, W_hh, b_ih, b_hh)
    res = run_bass_kernel_spmd(nc, in_maps, list(range(N_CORES)))
    return np.concatenate([r["out"] for r in res.results], axis=0)


# revision 40
# speedup vs baseline: 1.0853x; 1.0853x over previous
"""EpisodicMemory kernel for Trainium2, 8-core data-parallel.

Reference computation (per batch b, d=32, m=64 memory slots, 2 hops):
    M = vs[b]
    for hop:
        Rh[m,:] = R[b,hop,m] @ h[b,hop,m]                  # batched matvec
        z = [Rh*v, Rh*M, |Rh-v|, |Rh-M|]                   # [m, 4d]
        Z = tanh(z @ W1.T + b1) @ W2.T (+ b2: dropped — softmax-invariant)
        g = softmax(Z over m); o = sum_m ts[b,hop,m] * g[m]
        M = GRUCell(o, M)
    out[b] = M

Sharding: pure data parallel over batch; 128 batches per core.

Numerics: Rs/hs/ts are host-cast to bf16 (DMA bytes halve; DVE runs 2-byte
packed ops at 2x). Einsum reduce over e is an in-place bf16 add-tree on DVE
(TensorReduce gets no 2x mode; the tree does). Softmax and the GRU stay f32.

Per-core layout strategy:
  - einsum Rh: R tiles [128 part=(m,bp), free=(g4,d32,e32)] (b = blk*8+bp*4+g;
    p = m*2+bp, so one 128-partition DMA per block covers 1 MiB at full rate),
    mul (in-place, h broadcast over d) on DVE/Pool alternating, then a 5-level
    in-place add-tree over e on DVE.
  - features built in row layout [128 rows, (g,f,d)] bf16, PE-transposed to
    z^T [feat128, rows] for the MLP matmuls (bf16) on TensorE.
  - softmax/o batched per hop over all 128 batches [128 part=b, 64 m] in f32.
  - GRU in transposed layout [d part, b free], f32; M kept as MT [32,128],
    M_rep rebuilt in bf16 via a DRAM broadcast bounce.
"""

import numpy as np

import concourse.bacc as bacc
import concourse.bass as bass
import concourse.mybir as mybir
import concourse.tile as tile
from concourse.masks import make_identity
from concourse.tile import add_dep_helper

F32 = mybir.dt.float32
BF16 = mybir.dt.bfloat16
AF = mybir.ActivationFunctionType
ALU = mybir.AluOpType
AX = mybir.AxisListType

B, N_HOP, N_MEM, DIM = 1024, 2, 64, 32
N_CORES = 8
BC = B // N_CORES            # 128 batches per core
BB = 8                       # batches per block
NBLK = BC // BB              # 16 blocks
NG = BB // 2                 # 4 b-pair groups per block
ROWS = BB * N_MEM            # 512 rows per block
D4 = 4 * DIM                 # 128 MLP input features


def build_nc(n_iter: int = 1, variant: str = "full") -> bass.Bass:
    """variant: 'full' | 'dma' (loads only) | 'nodma' (R loaded once) |
    'nopool' (all muls on DVE) | 'allpool' (all muls on Pool)."""
    nc = bacc.Bacc("TRN2")

    # Rs/hs arrive host-permuted: [hop, blk, m, bp, g, ...] with b = blk*8+bp*4+g
    Rs_d = nc.dram_tensor(
        "Rs", [N_HOP, NBLK, N_MEM, 2, NG, DIM, DIM], BF16, kind="ExternalInput"
    )
    hs_d = nc.dram_tensor(
        "hs", [N_HOP, N_MEM, 2, NBLK, NG, DIM], BF16, kind="ExternalInput"
    )
    ts_d = nc.dram_tensor("ts", [BC, N_HOP, N_MEM, DIM], BF16, kind="ExternalInput")
    vs_d = nc.dram_tensor("vs", [BC, DIM], F32, kind="ExternalInput")
    W1_d = nc.dram_tensor("W1", [DIM, D4], F32, kind="ExternalInput")
    b1_d = nc.dram_tensor("b1", [DIM], F32, kind="ExternalInput")
    W2_d = nc.dram_tensor("W2", [1, DIM], F32, kind="ExternalInput")
    Wih_d = nc.dram_tensor("W_ih", [N_HOP, 3 * DIM, DIM], F32, kind="ExternalInput")
    Whh_d = nc.dram_tensor("W_hh", [N_HOP, 3 * DIM, DIM], F32, kind="ExternalInput")
    bih_d = nc.dram_tensor("b_ih", [N_HOP, 3 * DIM], F32, kind="ExternalInput")
    bhh_d = nc.dram_tensor("b_hh", [N_HOP, 3 * DIM], F32, kind="ExternalInput")
    out_d = nc.dram_tensor("out", [BC, DIM], F32, kind="ExternalOutput")
    # DRAM bounces for the v/M partition-broadcast (bf16)
    m_flat = nc.dram_tensor("m_flat", [BC, DIM], BF16)
    m_scr2 = nc.dram_tensor("m_scratch2", [2, NBLK, NG, DIM], BF16)
    m_scr3 = nc.dram_tensor("m_scratch3", [128, NBLK * NG * DIM], BF16)
    v_flat = nc.dram_tensor("v_flat", [BC, DIM], BF16)
    v_scr = nc.dram_tensor("v_scratch", [2, NBLK, NG, DIM], BF16)
    v_scr3 = nc.dram_tensor("v_scratch3", [128, NBLK * NG * DIM], BF16)
    # Z bounce, laid out so the gather is contiguous per natural batch index
    z_scr = nc.dram_tensor("z_scratch", [NBLK, 2, NG, N_MEM], F32)

    import contextlib

    with tile.TileContext(nc) as tc:
        with (
            (tc.For_i(0, n_iter, 1) if n_iter > 1 else contextlib.nullcontext()),
            tc.tile_pool(name="consts", bufs=1) as consts,
            tc.tile_pool(name="hop_io", bufs=2) as hop_io,
            tc.tile_pool(name="rpool", bufs=6) as rpool,
            tc.tile_pool(name="fpool", bufs=4) as fpool,
            tc.tile_pool(name="zpool", bufs=4) as zpool,
            tc.tile_pool(name="apool", bufs=4) as apool,
            tc.tile_pool(name="small", bufs=2) as small,
            tc.tile_pool(name="mstate", bufs=2) as mstate,
            tc.tile_pool(name="pp_z", bufs=3, space="PSUM") as pp_z,
            tc.tile_pool(name="pp_1", bufs=2, space="PSUM") as pp_1,
            tc.tile_pool(name="pp_2", bufs=1, space="PSUM") as pp_2,
            tc.tile_pool(name="pp_g", bufs=2, space="PSUM") as pp_g,
        ):
            ident = consts.tile([128, 128], F32)
            make_identity(nc, ident)
            ident16 = consts.tile([128, 128], BF16)
            nc.scalar.copy(out=ident16, in_=ident)

            # ---- weights prep (one-time) ----
            w1_sb = consts.tile([DIM, D4], F32)
            nc.sync.dma_start(out=w1_sb, in_=W1_d[:, :])
            w1t_ps = pp_g.tile([D4, DIM], F32, tag="gpsum")
            nc.tensor.transpose(w1t_ps, w1_sb, ident[:DIM, :DIM])
            W1T = consts.tile([D4, DIM], BF16)
            nc.scalar.copy(out=W1T, in_=w1t_ps)

            W2T_f = consts.tile([DIM, 1], F32)
            nc.sync.dma_start(out=W2T_f, in_=W2_d.rearrange("a b -> b a"))
            W2T = consts.tile([DIM, 1], BF16)
            nc.scalar.copy(out=W2T, in_=W2T_f)
            b1T = consts.tile([DIM, 1], F32)
            nc.sync.dma_start(out=b1T, in_=b1_d[:].unsqueeze(1))

            WihT, WhhT, bsum_rz, bihn_t, bhhn_t = [], [], [], [], []
            for hop in range(N_HOP):
                wih_sb = consts.tile([3 * DIM, DIM], F32, tag="wload", bufs=4)
                nc.sync.dma_start(out=wih_sb, in_=Wih_d[hop])
                wt_ps = pp_g.tile([DIM, 3 * DIM], F32, tag="gpsum")
                nc.tensor.transpose(wt_ps, wih_sb, ident[: 3 * DIM, : 3 * DIM])
                wT = consts.tile([DIM, 3 * DIM], F32, tag=f"wihT{hop}")
                nc.scalar.copy(out=wT, in_=wt_ps)
                WihT.append(wT)

                whh_sb = consts.tile([3 * DIM, DIM], F32, tag="wload", bufs=4)
                nc.sync.dma_start(out=whh_sb, in_=Whh_d[hop])
                wt_ps2 = pp_g.tile([DIM, 3 * DIM], F32, tag="gpsum")
                nc.tensor.transpose(wt_ps2, whh_sb, ident[: 3 * DIM, : 3 * DIM])
                wT2 = consts.tile([DIM, 3 * DIM], F32, tag=f"whhT{hop}")
                nc.scalar.copy(out=wT2, in_=wt_ps2)
                WhhT.append(wT2)

                # per-gate bias tiles, all at base partition 0
                gate_b = []
                for gd, gname in ((bih_d, "ih"), (bhh_d, "hh")):
                    for gate in range(3):
                        bt = consts.tile([DIM, 1], F32, tag=f"b{gname}{hop}{gate}")
                        nc.sync.dma_start(
                            out=bt,
                            in_=gd[hop, gate * DIM : (gate + 1) * DIM].unsqueeze(1),
                        )
                        gate_b.append(bt)
                b_r = consts.tile([DIM, 1], F32, tag=f"b_r{hop}")
                nc.vector.tensor_add(b_r, gate_b[0], gate_b[3])
                b_z = consts.tile([DIM, 1], F32, tag=f"b_z{hop}")
                nc.vector.tensor_add(b_z, gate_b[1], gate_b[4])
                bsum_rz.append((b_r, b_z))
                bihn_t.append(gate_b[2])
                bhhn_t.append(gate_b[5])

            # ---- initial M state ----
            vs_row = consts.tile([BC, DIM], F32)
            nc.sync.dma_start(out=vs_row, in_=vs_d[:, :])
            vst_ps = pp_g.tile([DIM, BC], F32, tag="gpsum")
            nc.tensor.transpose(vst_ps, vs_row, ident)
            vsT = consts.tile([DIM, BC], F32)
            nc.scalar.copy(out=vsT, in_=vst_ps)
            MT = vsT  # current M^T [d, b]

            # v_rep [128 part=(m,bp), (blk,g,d)] bf16: value vs[blk*8+bp*4+g, d].
            # Cast to bf16, stage a (bp, blk, g, d)-permuted copy in DRAM, then
            # replicate per-partition rows in DRAM (free-form APs), then a
            # plain [128, f] load (SBUF DMA APs must be partition-clean on HW).
            vs16 = consts.tile([BC, DIM], BF16)
            nc.scalar.copy(out=vs16, in_=vs_row)
            nc.sync.dma_start(out=v_flat[:, :], in_=vs16)
            nc.sync.dma_start(
                out=v_scr[:, :, :, :],
                in_=v_flat.rearrange("(blk bp g) d -> bp blk g d", bp=2, g=NG),
            )
            nc.sync.dma_start(
                out=v_scr3.rearrange("(m bp) f -> m bp f", bp=2),
                in_=v_scr.rearrange(
                    "bp blk g d -> bp (blk g d)"
                ).partition_broadcast(64),
            )
            v_rep = consts.tile([128, NBLK * NG * DIM], BF16)
            prev_bcast_dma = nc.sync.dma_start(out=v_rep, in_=v_scr3[:, :])

            M_rep = v_rep  # hop 0: M == vs

            for hop in range(N_HOP):
                # per-hop h in einsum layout [(bp,m), (blk,g,e)]
                # h for the whole hop: one contiguous [128, 2048] bf16 load
                h_hop = hop_io.tile([128, NBLK * NG * DIM], BF16, tag="h_hop")
                nc.scalar.dma_start(
                    out=h_hop,
                    in_=hs_d[hop].rearrange("m bp blk g e -> (m bp) (blk g e)"),
                )
                # t natural layout [b, (m,d)]
                t_hop = hop_io.tile([BC, N_MEM * DIM], BF16, tag="t_hop")
                t_dma = nc.scalar.dma_start(
                    out=t_hop, in_=ts_d[:, hop].rearrange("b m d -> b (m d)")
                )
                # lane-ordering: keep the broadcast DMA strictly before t_hop
                add_dep_helper(t_dma.ins, prev_bcast_dma.ins,
                               reason="hwdge lane ordering")

                Z_row = small.tile([BC, N_MEM], F32, tag="Z_row")

                for blk in range(NBLK):
                    if variant == "nodma":
                        if hop == 0 and blk == 0:
                            r_tile = consts.tile(
                                [128, NG * DIM * DIM], BF16, tag="Rconst"
                            )
                            nc.sync.dma_start(
                                out=r_tile,
                                in_=Rs_d[hop, blk].rearrange(
                                    "m bp g d e -> (m bp) (g d e)"
                                ),
                            )
                            r_const = r_tile
                        r_tile = rpool.tile([128, NG * DIM * DIM], BF16, tag="R")
                        nc.vector.tensor_copy(r_tile, r_const)
                    else:
                        r_tile = rpool.tile([128, NG * DIM * DIM], BF16, tag="R")
                        nc.sync.dma_start(
                            out=r_tile,
                            in_=Rs_d[hop, blk].rearrange(
                                "m bp g d e -> (m bp) (g d e)"
                            ),
                        )
                    if variant == "dma":
                        continue
                    # P = R * h (in-place), h broadcast over d
                    r4 = r_tile.rearrange("p (g d e) -> p g d e", g=NG, d=DIM)
                    h_slice = h_hop[
                        :, blk * NG * DIM : (blk + 1) * NG * DIM
                    ].rearrange("p (g e) -> p g e", g=NG)
                    # odd blocks: broadcast-mul on Pool/GPSIMD (engine-time
                    # indifferent to the stride-0 operand) so it overlaps DVE.
                    on_pool = blk % 2 == 1
                    if variant == "nopool":
                        on_pool = False
                    elif variant == "allpool":
                        on_pool = True
                    if on_pool:
                        h_v = h_slice.unsqueeze(2).broadcast_to(
                            (128, NG, DIM, DIM)
                        )
                        nc.gpsimd.tensor_tensor(r4, r4, h_v, op=ALU.mult)
                    else:
                        # DVE: the stride-0 broadcast operand defeats the DVE
                        # 2-byte fast path on HW, so materialize h replicated
                        # over d via log-doubling packed copies, then a fully
                        # packed mul.
                        hr = fpool.tile([128, NG * DIM * DIM], BF16, tag="hrep")
                        h4 = hr.rearrange("p (g d e) -> p g d e", g=NG, d=DIM)
                        nc.vector.tensor_copy(
                            h4[:, :, 0:1, :], h_slice.unsqueeze(2)
                        )
                        for dlo, dhi in ((1, 2), (2, 4), (4, 8), (8, 16), (16, 32)):
                            nc.vector.tensor_copy(
                                h4[:, :, dlo:dhi, :], h4[:, :, 0 : dhi - dlo, :]
                            )
                        nc.vector.tensor_mul(r_tile, r_tile, hr)
                    # Rh[(bp,m), (g,d)] = sum_e P via in-place bf16 add-tree
                    # (TensorReduce gets no 2-byte 2x mode; packed adds do)
                    p3 = r_tile.rearrange("p (gd e) -> p gd e", e=DIM)
                    for half in (16, 8, 4, 2):
                        nc.vector.tensor_add(
                            p3[:, :, :half], p3[:, :, :half], p3[:, :, half : 2 * half]
                        )
                    rh = fpool.tile([128, NG * DIM], BF16, tag="rh")
                    nc.vector.tensor_add(
                        rh.rearrange("p (gd o) -> p gd o", o=1),
                        p3[:, :, 0:1],
                        p3[:, :, 1:2],
                    )
                    # features F [(bp,m), (g, f, d)] bf16
                    f_blk = fpool.tile([128, NG * 4 * DIM], BF16, tag="F")
                    f4 = f_blk.rearrange("p (g f d) -> p g f d", g=NG, f=4)
                    rh3 = rh.rearrange("p (g d) -> p g d", g=NG)
                    vr3 = v_rep[:, blk * NG * DIM : (blk + 1) * NG * DIM].rearrange(
                        "p (g d) -> p g d", g=NG
                    )
                    mr3 = M_rep[:, blk * NG * DIM : (blk + 1) * NG * DIM].rearrange(
                        "p (g d) -> p g d", g=NG
                    )
                    nc.vector.tensor_mul(f4[:, :, 0, :], rh3, vr3)
                    nc.vector.tensor_mul(f4[:, :, 1, :], rh3, mr3)
                    nc.vector.tensor_sub(f4[:, :, 2, :], rh3, vr3)
                    nc.vector.tensor_sub(f4[:, :, 3, :], rh3, mr3)
                    nc.scalar.activation(f4[:, :, 2, :], f4[:, :, 2, :], AF.Abs)
                    nc.scalar.activation(f4[:, :, 3, :], f4[:, :, 3, :], AF.Abs)

                    # transpose to z^T [(f,d), (g,bp,m)]
                    zt_ps = pp_z.tile([D4, ROWS], BF16, tag="zt")
                    for g in range(NG):
                        nc.tensor.transpose(
                            zt_ps[:, g * 128 : (g + 1) * 128],
                            f_blk[:, g * 128 : (g + 1) * 128],
                            ident16,
                        )
                    zt_sb = zpool.tile([D4, ROWS], BF16, tag="zt_sb")
                    nc.scalar.copy(out=zt_sb, in_=zt_ps)

                    ps1 = pp_1.tile([DIM, ROWS], F32, tag="ps1")
                    nc.tensor.matmul(ps1, lhsT=W1T, rhs=zt_sb, start=True, stop=True)
                    a1 = apool.tile([DIM, ROWS], BF16, tag="a1")
                    nc.scalar.activation(a1, ps1, AF.Tanh, bias=b1T)
                    ps2 = pp_2.tile([1, ROWS], F32, tag="ps2")
                    nc.tensor.matmul(ps2, lhsT=W2T, rhs=a1, start=True, stop=True)
                    z_sb = zpool.tile([1, ROWS], F32, tag="z_sb")
                    nc.scalar.copy(out=z_sb, in_=ps2)
                    # z_sb free order is (g, m, bp); store as (bp, g, m).
                    # src stays 1-partition (dim0 count 1): split by bp.
                    for bp in range(2):
                        nc.scalar.dma_start(
                            out=z_scr[blk, bp].unsqueeze(0),
                            in_=z_sb.rearrange("o (g m bp) -> o g m bp", g=NG, bp=2)[
                                :, :, :, bp
                            ],
                        )

                # gather Z rows from DRAM: flat (blk,bp,g) == natural b
                nc.scalar.dma_start(
                    out=Z_row,
                    in_=z_scr.rearrange("a b c m -> (a b c) m"),
                )

                # softmax over m, batched over all 128 b. |Z| is small
                # (tanh-bounded second layer), so skip the max-subtract and
                # normalize o AFTER the t-reduction: o = (sum_m t*e) / sum_m e
                e_row = small.tile([BC, N_MEM], F32, tag="e_row")
                nc.scalar.activation(e_row, Z_row, AF.Exp)
                e16 = small.tile([BC, N_MEM], BF16, tag="e16")
                nc.scalar.copy(out=e16, in_=e_row)
                ssum = small.tile([BC, 1], F32, tag="ssum")
                nc.vector.tensor_reduce(out=ssum, in_=e_row, axis=AX.X, op=ALU.add)
                rsum = small.tile([BC, 1], F32, tag="rsum")
                nc.vector.reciprocal(rsum, ssum)

                # o[b,d] = (sum_m t[b,m,d] * e[b,m]) * rsum[b].
                # Expand e over d via packed doubling (broadcast APs defeat
                # the DVE 2-byte fast path on HW), packed mul, then a packed
                # add-tree over m.
                e_rep = small.tile([BC, N_MEM * DIM], BF16, tag="e_rep")
                er3 = e_rep.rearrange("b (m d) -> b m d", d=DIM)
                nc.vector.tensor_copy(er3[:, :, 0:1], e16.unsqueeze(2))
                for dlo, dhi in ((1, 2), (2, 4), (4, 8), (8, 16), (16, 32)):
                    nc.vector.tensor_copy(
                        er3[:, :, dlo:dhi], er3[:, :, 0 : dhi - dlo]
                    )
                t3 = t_hop.rearrange("b (m d) -> b m d", d=DIM)
                nc.vector.tensor_mul(t_hop, t_hop, e_rep)
                for mh in (32, 16, 8, 4, 2):
                    nc.vector.tensor_add(
                        t3[:, :mh, :], t3[:, :mh, :], t3[:, mh : 2 * mh, :]
                    )
                o_raw = small.tile([BC, DIM], F32, tag="o_raw")
                nc.vector.tensor_add(
                    o_raw.unsqueeze(1), t3[:, 0:1, :], t3[:, 1:2, :]
                )
                o_row = small.tile([BC, DIM], F32, tag="o_row")
                nc.vector.tensor_scalar_mul(o_row, o_raw, rsum)

                # GRU (transposed layout [*, b], f32)
                ot_ps = pp_g.tile([DIM, BC], F32, tag="gpsum")
                nc.tensor.transpose(ot_ps, o_row, ident)
                oT = small.tile([DIM, BC], F32, tag="oT")
                nc.scalar.copy(out=oT, in_=ot_ps)

                # per-gate matmuls so every gate tile sits at base partition 0
                def gate_pair(g):
                    gi = pp_g.tile([DIM, BC], F32, tag="gpsum")
                    nc.tensor.matmul(
                        gi,
                        lhsT=WihT[hop][:, g * DIM : (g + 1) * DIM],
                        rhs=oT,
                        start=True,
                        stop=True,
                    )
                    gh = pp_g.tile([DIM, BC], F32, tag="gpsum")
                    nc.tensor.matmul(
                        gh,
                        lhsT=WhhT[hop][:, g * DIM : (g + 1) * DIM],
                        rhs=MT,
                        start=True,
                        stop=True,
                    )
                    return gi, gh

                # r,z gates: sigmoid(gi + gh + b_ih + b_hh)
                rz_t = []
                for g in range(2):
                    gi, gh = gate_pair(g)
                    gb = small.tile([DIM, BC], F32, tag=f"g{g}b")
                    nc.scalar.activation(gb, gi, AF.Identity, bias=bsum_rz[hop][g])
                    nc.vector.tensor_add(gb, gb, gh)
                    gt = small.tile([DIM, BC], F32, tag=f"gate{g}")
                    nc.scalar.activation(gt, gb, AF.Sigmoid)
                    rz_t.append(gt)
                r_t, z_t = rz_t

                # n = tanh(gi_n + b_ih_n + r * (gh_n + b_hh_n))
                gi_n, gh_n = gate_pair(2)
                ghn = small.tile([DIM, BC], F32, tag="ghn")
                nc.scalar.activation(ghn, gh_n, AF.Identity, bias=bhhn_t[hop])
                gin = small.tile([DIM, BC], F32, tag="gin")
                nc.scalar.activation(gin, gi_n, AF.Identity, bias=bihn_t[hop])
                n1 = small.tile([DIM, BC], F32, tag="n1")
                nc.vector.tensor_mul(n1, r_t, ghn)
                nc.vector.tensor_add(n1, n1, gin)
                n_t = small.tile([DIM, BC], F32, tag="n_t")
                nc.scalar.activation(n_t, n1, AF.Tanh)

                # M' = n + z * (M - n)
                MT_new = mstate.tile([DIM, BC], F32, tag="MT")
                nc.vector.tensor_sub(MT_new, MT, n_t)
                nc.vector.tensor_mul(MT_new, MT_new, z_t)
                nc.vector.tensor_add(MT_new, MT_new, n_t)
                MT = MT_new

                if hop < N_HOP - 1:
                    # rebuild M_rep (bf16) via DRAM bounce
                    mrow_ps = pp_g.tile([BC, DIM], F32, tag="gpsum")
                    nc.tensor.transpose(mrow_ps, MT, ident[:DIM, :DIM])
                    M16 = mstate.tile([BC, DIM], BF16, tag="M16")
                    nc.scalar.copy(out=M16, in_=mrow_ps)
                    nc.scalar.dma_start(out=m_flat[:, :], in_=M16)
                    nc.sync.dma_start(
                        out=m_scr2[:, :, :, :],
                        in_=m_flat.rearrange(
                            "(blk bp g) d -> bp blk g d", bp=2, g=NG
                        ),
                    )
                    M_rep_new = mstate.tile(
                        [128, NBLK * NG * DIM], BF16, tag="M_rep", bufs=1
                    )
                    nc.sync.dma_start(
                        out=m_scr3.rearrange("(m bp) f -> m bp f", bp=2),
                        in_=m_scr2.rearrange(
                            "bp blk g d -> bp (blk g d)"
                        ).partition_broadcast(64),
                    )
                    prev_bcast_dma = nc.sync.dma_start(
                        out=M_rep_new, in_=m_scr3[:, :]
                    )
                    M_rep = M_rep_new
                else:
                    # row-major M only needed for the final output
                    mrow_ps = pp_g.tile([BC, DIM], F32, tag="gpsum")
                    nc.tensor.transpose(mrow_ps, MT, ident[:DIM, :DIM])
                    M_row = mstate.tile([BC, DIM], F32, tag="M_row")
                    nc.scalar.copy(out=M_row, in_=mrow_ps)
                    nc.sync.dma_start(out=out_d[:, :], in_=M_row)

    nc.compile()
    return nc


_NC_CACHE = None


def _get_nc():
    global _NC_CACHE
    if _NC_CACHE is None:
        _NC_CACHE = build_nc()
    return _NC_CACHE


def _bf16(x):
    import ml_dtypes

    return np.asarray(x).astype(ml_dtypes.bfloat16)


def permute_local(x):
    """[BC, N_HOP, m, ...] -> [N_HOP, NBLK, m, 2, NG, ...] with b = blk*8+bp*4+g."""
    tail = x.shape[2:]
    y = x.reshape(NBLK, 2, NG, N_HOP, *tail)
    order = (3, 0, 4, 1, 2) + tuple(range(5, y.ndim))
    return np.ascontiguousarray(y.transpose(order))


def permute_h(x):
    """hs [BC, N_HOP, m, e] -> [N_HOP, m, 2, NBLK, NG, e]."""
    y = x.reshape(NBLK, 2, NG, N_HOP, N_MEM, DIM)
    return np.ascontiguousarray(y.transpose(3, 4, 1, 0, 2, 5))


def make_in_maps(hs, Rs, ts, vs, W1, b1, W2, W_ih, W_hh, b_ih, b_hh):
    in_maps = []
    for c in range(N_CORES):
        sl = slice(c * BC, (c + 1) * BC)
        in_maps.append(
            {
                "Rs": permute_local(_bf16(Rs[sl])),
                "hs": permute_h(_bf16(hs[sl])),
                "ts": _bf16(ts[sl]),
                "vs": np.ascontiguousarray(vs[sl]),
                "W1": np.ascontiguousarray(W1),
                "b1": np.ascontiguousarray(b1),
                "W2": np.ascontiguousarray(W2),
                "W_ih": np.ascontiguousarray(W_ih),
                "W_hh": np.ascontiguousarray(W_hh),
                "b_ih": np.ascontiguousarray(b_ih),
                "b_hh": np.ascontiguousarray(b_hh),
            }
        )
    return in_maps


def kernel(hs, Rs, ts, vs, W1, b1, W2, b2, W_ih, W_hh, b_ih, b_hh):
    from concourse.bass_utils import run_bass_kernel_spmd

    nc = _get_nc()
    in_maps = make_in_maps(hs, Rs, ts, vs, W1, b1, W2, W_ih, W_hh, b_ih, b_hh)
    res = run_bass_kernel_spmd(nc, in_maps, list(range(N_CORES)))
    return np.concatenate([r["out"] for r in res.results], axis=0)


# revision 41
# speedup vs baseline: 1.1192x; 1.0312x over previous
"""EpisodicMemory kernel for Trainium2, 8-core data-parallel.

Reference computation (per batch b, d=32, m=64 memory slots, 2 hops):
    M = vs[b]
    for hop:
        Rh[m,:] = R[b,hop,m] @ h[b,hop,m]                  # batched matvec
        z = [Rh*v, Rh*M, |Rh-v|, |Rh-M|]                   # [m, 4d]
        Z = tanh(z @ W1.T + b1) @ W2.T (+ b2: dropped — softmax-invariant)
        g = softmax(Z over m); o = sum_m ts[b,hop,m] * g[m]
        M = GRUCell(o, M)
    out[b] = M

Sharding: pure data parallel over batch; 128 batches per core.

Numerics: Rs/hs/ts are host-cast to bf16 (DMA bytes halve; DVE runs 2-byte
packed ops at 2x). Einsum reduce over e is an in-place bf16 add-tree on DVE
(TensorReduce gets no 2x mode; the tree does). Softmax and the GRU stay f32.

Per-core layout strategy:
  - einsum Rh: R tiles [128 part=(m,bp), free=(g4,d32,e32)] (b = blk*8+bp*4+g;
    p = m*2+bp, so one 128-partition DMA per block covers 1 MiB at full rate),
    mul (in-place, h broadcast over d) on DVE/Pool alternating, then a 5-level
    in-place add-tree over e on DVE.
  - features built in row layout [128 rows, (g,f,d)] bf16, PE-transposed to
    z^T [feat128, rows] for the MLP matmuls (bf16) on TensorE.
  - softmax/o batched per hop over all 128 batches [128 part=b, 64 m] in f32.
  - GRU in transposed layout [d part, b free], f32; M kept as MT [32,128],
    M_rep rebuilt in bf16 via a DRAM broadcast bounce.
"""

import numpy as np

import concourse.bacc as bacc
import concourse.bass as bass
import concourse.mybir as mybir
import concourse.tile as tile
from concourse.masks import make_identity
from concourse.tile import add_dep_helper

F32 = mybir.dt.float32
BF16 = mybir.dt.bfloat16
AF = mybir.ActivationFunctionType
ALU = mybir.AluOpType
AX = mybir.AxisListType

B, N_HOP, N_MEM, DIM = 1024, 2, 64, 32
N_CORES = 8
BC = B // N_CORES            # 128 batches per core
BB = 8                       # batches per block
NBLK = BC // BB              # 16 blocks
NG = BB // 2                 # 4 b-pair groups per block
ROWS = BB * N_MEM            # 512 rows per block
D4 = 4 * DIM                 # 128 MLP input features


def build_nc(n_iter: int = 1, variant: str = "full") -> bass.Bass:
    """variant: 'full' | 'dma' (loads only) | 'nodma' (R loaded once) |
    'nopool' (all muls on DVE) | 'allpool' (all muls on Pool)."""
    nc = bacc.Bacc("TRN2")

    # Rs/hs arrive host-permuted: [hop, blk, m, bp, g, ...] with b = blk*8+bp*4+g
    Rs_d = nc.dram_tensor(
        "Rs", [N_HOP, NBLK, N_MEM, 2, NG, DIM, DIM], BF16, kind="ExternalInput"
    )
    hs_d = nc.dram_tensor(
        "hs", [N_HOP, N_MEM, 2, NBLK, NG, DIM], BF16, kind="ExternalInput"
    )
    ts_d = nc.dram_tensor("ts", [BC, N_HOP, N_MEM, DIM], BF16, kind="ExternalInput")
    vs_d = nc.dram_tensor("vs", [BC, DIM], F32, kind="ExternalInput")
    W1_d = nc.dram_tensor("W1", [DIM, D4], F32, kind="ExternalInput")
    b1_d = nc.dram_tensor("b1", [DIM], F32, kind="ExternalInput")
    W2_d = nc.dram_tensor("W2", [1, DIM], F32, kind="ExternalInput")
    Wih_d = nc.dram_tensor("W_ih", [N_HOP, 3 * DIM, DIM], F32, kind="ExternalInput")
    Whh_d = nc.dram_tensor("W_hh", [N_HOP, 3 * DIM, DIM], F32, kind="ExternalInput")
    bih_d = nc.dram_tensor("b_ih", [N_HOP, 3 * DIM], F32, kind="ExternalInput")
    bhh_d = nc.dram_tensor("b_hh", [N_HOP, 3 * DIM], F32, kind="ExternalInput")
    out_d = nc.dram_tensor("out", [BC, DIM], F32, kind="ExternalOutput")
    # DRAM bounces for the v/M partition-broadcast (bf16)
    m_flat = nc.dram_tensor("m_flat", [BC, DIM], BF16)
    m_scr2 = nc.dram_tensor("m_scratch2", [2, NBLK, NG, DIM], BF16)
    m_scr3 = nc.dram_tensor("m_scratch3", [128, NBLK * NG * DIM], BF16)
    v_flat = nc.dram_tensor("v_flat", [BC, DIM], BF16)
    v_scr = nc.dram_tensor("v_scratch", [2, NBLK, NG, DIM], BF16)
    v_scr3 = nc.dram_tensor("v_scratch3", [128, NBLK * NG * DIM], BF16)
    # Z bounce, laid out so the gather is contiguous per natural batch index
    z_scr = nc.dram_tensor("z_scratch", [NBLK, 2, NG, N_MEM], F32)

    import contextlib

    with tile.TileContext(nc) as tc:
        with (
            (tc.For_i(0, n_iter, 1) if n_iter > 1 else contextlib.nullcontext()),
            tc.tile_pool(name="consts", bufs=1) as consts,
            tc.tile_pool(name="hop_io", bufs=2) as hop_io,
            tc.tile_pool(name="rpool", bufs=6) as rpool,
            tc.tile_pool(name="fpool", bufs=4) as fpool,
            tc.tile_pool(name="zpool", bufs=4) as zpool,
            tc.tile_pool(name="apool", bufs=4) as apool,
            tc.tile_pool(name="small", bufs=2) as small,
            tc.tile_pool(name="mstate", bufs=2) as mstate,
            tc.tile_pool(name="pp_z", bufs=3, space="PSUM") as pp_z,
            tc.tile_pool(name="pp_1", bufs=2, space="PSUM") as pp_1,
            tc.tile_pool(name="pp_2", bufs=1, space="PSUM") as pp_2,
            tc.tile_pool(name="pp_g", bufs=2, space="PSUM") as pp_g,
        ):
            ident = consts.tile([128, 128], F32)
            make_identity(nc, ident)
            ident16 = consts.tile([128, 128], BF16)
            nc.scalar.copy(out=ident16, in_=ident)

            # ---- weights prep (one-time) ----
            w1_sb = consts.tile([DIM, D4], F32)
            nc.sync.dma_start(out=w1_sb, in_=W1_d[:, :])
            w1t_ps = pp_g.tile([D4, DIM], F32, tag="gpsum")
            nc.tensor.transpose(w1t_ps, w1_sb, ident[:DIM, :DIM])
            W1T = consts.tile([D4, DIM], BF16)
            nc.scalar.copy(out=W1T, in_=w1t_ps)

            W2T_f = consts.tile([DIM, 1], F32)
            nc.sync.dma_start(out=W2T_f, in_=W2_d.rearrange("a b -> b a"))
            W2T = consts.tile([DIM, 1], BF16)
            nc.scalar.copy(out=W2T, in_=W2T_f)
            b1T = consts.tile([DIM, 1], F32)
            nc.sync.dma_start(out=b1T, in_=b1_d[:].unsqueeze(1))

            WihT, WhhT, bsum_rz, bihn_t, bhhn_t = [], [], [], [], []
            for hop in range(N_HOP):
                wih_sb = consts.tile([3 * DIM, DIM], F32, tag="wload", bufs=4)
                nc.sync.dma_start(out=wih_sb, in_=Wih_d[hop])
                wt_ps = pp_g.tile([DIM, 3 * DIM], F32, tag="gpsum")
                nc.tensor.transpose(wt_ps, wih_sb, ident[: 3 * DIM, : 3 * DIM])
                wT = consts.tile([DIM, 3 * DIM], F32, tag=f"wihT{hop}")
                nc.scalar.copy(out=wT, in_=wt_ps)
                WihT.append(wT)

                whh_sb = consts.tile([3 * DIM, DIM], F32, tag="wload", bufs=4)
                nc.sync.dma_start(out=whh_sb, in_=Whh_d[hop])
                wt_ps2 = pp_g.tile([DIM, 3 * DIM], F32, tag="gpsum")
                nc.tensor.transpose(wt_ps2, whh_sb, ident[: 3 * DIM, : 3 * DIM])
                wT2 = consts.tile([DIM, 3 * DIM], F32, tag=f"whhT{hop}")
                nc.scalar.copy(out=wT2, in_=wt_ps2)
                WhhT.append(wT2)

                # per-gate bias tiles, all at base partition 0
                gate_b = []
                for gd, gname in ((bih_d, "ih"), (bhh_d, "hh")):
                    for gate in range(3):
                        bt = consts.tile([DIM, 1], F32, tag=f"b{gname}{hop}{gate}")
                        nc.sync.dma_start(
                            out=bt,
                            in_=gd[hop, gate * DIM : (gate + 1) * DIM].unsqueeze(1),
                        )
                        gate_b.append(bt)
                b_r = consts.tile([DIM, 1], F32, tag=f"b_r{hop}")
                nc.vector.tensor_add(b_r, gate_b[0], gate_b[3])
                b_z = consts.tile([DIM, 1], F32, tag=f"b_z{hop}")
                nc.vector.tensor_add(b_z, gate_b[1], gate_b[4])
                bsum_rz.append((b_r, b_z))
                bihn_t.append(gate_b[2])
                bhhn_t.append(gate_b[5])

            # ---- initial M state ----
            vs_row = consts.tile([BC, DIM], F32)
            nc.sync.dma_start(out=vs_row, in_=vs_d[:, :])
            vst_ps = pp_g.tile([DIM, BC], F32, tag="gpsum")
            nc.tensor.transpose(vst_ps, vs_row, ident)
            vsT = consts.tile([DIM, BC], F32)
            nc.scalar.copy(out=vsT, in_=vst_ps)
            MT = vsT  # current M^T [d, b]

            # v_rep [128 part=(m,bp), (blk,g,d)] bf16: value vs[blk*8+bp*4+g, d].
            # Cast to bf16, stage a (bp, blk, g, d)-permuted copy in DRAM, then
            # replicate per-partition rows in DRAM (free-form APs), then a
            # plain [128, f] load (SBUF DMA APs must be partition-clean on HW).
            vs16 = consts.tile([BC, DIM], BF16)
            nc.scalar.copy(out=vs16, in_=vs_row)
            nc.sync.dma_start(out=v_flat[:, :], in_=vs16)
            nc.sync.dma_start(
                out=v_scr[:, :, :, :],
                in_=v_flat.rearrange("(blk bp g) d -> bp blk g d", bp=2, g=NG),
            )
            nc.sync.dma_start(
                out=v_scr3.rearrange("(m bp) f -> m bp f", bp=2),
                in_=v_scr.rearrange(
                    "bp blk g d -> bp (blk g d)"
                ).partition_broadcast(64),
            )
            v_rep = consts.tile([128, NBLK * NG * DIM], BF16)
            prev_bcast_dma = nc.sync.dma_start(out=v_rep, in_=v_scr3[:, :])

            M_rep = v_rep  # hop 0: M == vs

            for hop in range(N_HOP):
                # per-hop h in einsum layout [(bp,m), (blk,g,e)]
                # h for the whole hop: one contiguous [128, 2048] bf16 load
                h_hop = hop_io.tile([128, NBLK * NG * DIM], BF16, tag="h_hop")
                nc.scalar.dma_start(
                    out=h_hop,
                    in_=hs_d[hop].rearrange("m bp blk g e -> (m bp) (blk g e)"),
                )
                # t natural layout [b, (m,d)]
                t_hop = hop_io.tile([BC, N_MEM * DIM], BF16, tag="t_hop")
                t_dma = nc.scalar.dma_start(
                    out=t_hop, in_=ts_d[:, hop].rearrange("b m d -> b (m d)")
                )
                # lane-ordering: keep the broadcast DMA strictly before t_hop
                add_dep_helper(t_dma.ins, prev_bcast_dma.ins,
                               reason="hwdge lane ordering")

                Z_row = small.tile([BC, N_MEM], F32, tag="Z_row")

                for blk in range(NBLK):
                    if variant == "nodma":
                        if hop == 0 and blk == 0:
                            r_tile = consts.tile(
                                [128, NG * DIM * DIM], BF16, tag="Rconst"
                            )
                            nc.sync.dma_start(
                                out=r_tile,
                                in_=Rs_d[hop, blk].rearrange(
                                    "m bp g d e -> (m bp) (g d e)"
                                ),
                            )
                            r_const = r_tile
                        r_tile = rpool.tile([128, NG * DIM * DIM], BF16, tag="R")
                        nc.vector.tensor_copy(r_tile, r_const)
                    else:
                        r_tile = rpool.tile([128, NG * DIM * DIM], BF16, tag="R")
                        nc.sync.dma_start(
                            out=r_tile,
                            in_=Rs_d[hop, blk].rearrange(
                                "m bp g d e -> (m bp) (g d e)"
                            ),
                        )
                    if variant == "dma":
                        continue
                    # P = R * h (in-place), h broadcast over d
                    r4 = r_tile.rearrange("p (g d e) -> p g d e", g=NG, d=DIM)
                    h_slice = h_hop[
                        :, blk * NG * DIM : (blk + 1) * NG * DIM
                    ].rearrange("p (g e) -> p g e", g=NG)
                    # odd blocks: broadcast-mul on Pool/GPSIMD (engine-time
                    # indifferent to the stride-0 operand) so it overlaps DVE.
                    on_pool = blk % 2 == 1
                    if variant == "nopool":
                        on_pool = False
                    elif variant == "allpool":
                        on_pool = True
                    if on_pool:
                        h_v = h_slice.unsqueeze(2).broadcast_to(
                            (128, NG, DIM, DIM)
                        )
                        nc.gpsimd.tensor_tensor(r4, r4, h_v, op=ALU.mult)
                    else:
                        # DVE: the stride-0 broadcast operand defeats the DVE
                        # 2-byte fast path on HW, so materialize h replicated
                        # over d via log-doubling packed copies, then a fully
                        # packed mul.
                        hr = fpool.tile([128, NG * DIM * DIM], BF16, tag="hrep")
                        h4 = hr.rearrange("p (g d e) -> p g d e", g=NG, d=DIM)
                        nc.vector.tensor_copy(
                            h4[:, :, 0:1, :], h_slice.unsqueeze(2)
                        )
                        for dlo, dhi in ((1, 2), (2, 4), (4, 8), (8, 16), (16, 32)):
                            nc.vector.tensor_copy(
                                h4[:, :, dlo:dhi, :], h4[:, :, 0 : dhi - dlo, :]
                            )
                        nc.vector.tensor_mul(r_tile, r_tile, hr)
                    # Rh[(bp,m), (g,d)] = sum_e P via in-place bf16 add-tree
                    # (TensorReduce gets no 2-byte 2x mode; packed adds do)
                    p3 = r_tile.rearrange("p (gd e) -> p gd e", e=DIM)
                    for half in (16, 8, 4, 2):
                        nc.vector.tensor_add(
                            p3[:, :, :half], p3[:, :, :half], p3[:, :, half : 2 * half]
                        )
                    rh = fpool.tile([128, NG * DIM], BF16, tag="rh")
                    nc.vector.tensor_add(
                        rh.rearrange("p (gd o) -> p gd o", o=1),
                        p3[:, :, 0:1],
                        p3[:, :, 1:2],
                    )
                    # features F [(bp,m), (g, f, d)] bf16
                    f_blk = fpool.tile([128, NG * 4 * DIM], BF16, tag="F")
                    f4 = f_blk.rearrange("p (g f d) -> p g f d", g=NG, f=4)
                    rh3 = rh.rearrange("p (g d) -> p g d", g=NG)
                    vr3 = v_rep[:, blk * NG * DIM : (blk + 1) * NG * DIM].rearrange(
                        "p (g d) -> p g d", g=NG
                    )
                    mr3 = M_rep[:, blk * NG * DIM : (blk + 1) * NG * DIM].rearrange(
                        "p (g d) -> p g d", g=NG
                    )
                    nc.vector.tensor_mul(f4[:, :, 0, :], rh3, vr3)
                    nc.vector.tensor_mul(f4[:, :, 1, :], rh3, mr3)
                    nc.vector.tensor_sub(f4[:, :, 2, :], rh3, vr3)
                    nc.vector.tensor_sub(f4[:, :, 3, :], rh3, mr3)
                    nc.scalar.activation(f4[:, :, 2, :], f4[:, :, 2, :], AF.Abs)
                    nc.scalar.activation(f4[:, :, 3, :], f4[:, :, 3, :], AF.Abs)

                    # transpose to z^T [(f,d), (g,bp,m)]
                    zt_ps = pp_z.tile([D4, ROWS], BF16, tag="zt")
                    for g in range(NG):
                        nc.tensor.transpose(
                            zt_ps[:, g * 128 : (g + 1) * 128],
                            f_blk[:, g * 128 : (g + 1) * 128],
                            ident16,
                        )
                    zt_sb = zpool.tile([D4, ROWS], BF16, tag="zt_sb")
                    nc.scalar.copy(out=zt_sb, in_=zt_ps)

                    ps1 = pp_1.tile([DIM, ROWS], F32, tag="ps1")
                    nc.tensor.matmul(ps1, lhsT=W1T, rhs=zt_sb, start=True, stop=True)
                    a1 = apool.tile([DIM, ROWS], BF16, tag="a1")
                    nc.scalar.activation(a1, ps1, AF.Tanh, bias=b1T)
                    ps2 = pp_2.tile([1, ROWS], F32, tag="ps2")
                    nc.tensor.matmul(ps2, lhsT=W2T, rhs=a1, start=True, stop=True)
                    z_sb = zpool.tile([1, ROWS], F32, tag="z_sb")
                    nc.scalar.copy(out=z_sb, in_=ps2)
                    # z_sb free order is (g, m, bp); store as (bp, g, m).
                    # src stays 1-partition (dim0 count 1): split by bp.
                    for bp in range(2):
                        nc.scalar.dma_start(
                            out=z_scr[blk, bp].unsqueeze(0),
                            in_=z_sb.rearrange("o (g m bp) -> o g m bp", g=NG, bp=2)[
                                :, :, :, bp
                            ],
                        )

                # gather Z rows from DRAM: flat (blk,bp,g) == natural b
                nc.scalar.dma_start(
                    out=Z_row,
                    in_=z_scr.rearrange("a b c m -> (a b c) m"),
                )

                # softmax over m, batched over all 128 b. |Z| is small
                # (tanh-bounded second layer), so skip the max-subtract and
                # normalize o AFTER the t-reduction: o = (sum_m t*e) / sum_m e
                e_row = small.tile([BC, N_MEM], F32, tag="e_row")
                nc.scalar.activation(e_row, Z_row, AF.Exp)
                e16 = small.tile([BC, N_MEM], BF16, tag="e16")
                nc.scalar.copy(out=e16, in_=e_row)
                ssum = small.tile([BC, 1], F32, tag="ssum")
                nc.vector.tensor_reduce(out=ssum, in_=e_row, axis=AX.X, op=ALU.add)
                rsum = small.tile([BC, 1], F32, tag="rsum")
                nc.vector.reciprocal(rsum, ssum)

                # o[b,d] = (sum_m t[b,m,d] * e[b,m]) * rsum[b]
                t3 = t_hop.rearrange("b (m d) -> b m d", d=DIM)
                g3 = e16.unsqueeze(2).broadcast_to((BC, N_MEM, DIM))
                nc.vector.tensor_mul(t3, t3, g3)
                o_raw = small.tile([BC, DIM], F32, tag="o_raw")
                nc.vector.tensor_reduce(
                    out=o_raw,
                    in_=t_hop.rearrange("b (m d) -> b d m", d=DIM),
                    axis=AX.X,
                    op=ALU.add,
                )
                o_row = small.tile([BC, DIM], F32, tag="o_row")
                nc.vector.tensor_scalar_mul(o_row, o_raw, rsum)

                # GRU (transposed layout [*, b], f32)
                ot_ps = pp_g.tile([DIM, BC], F32, tag="gpsum")
                nc.tensor.transpose(ot_ps, o_row, ident)
                oT = small.tile([DIM, BC], F32, tag="oT")
                nc.scalar.copy(out=oT, in_=ot_ps)

                # per-gate matmuls so every gate tile sits at base partition 0
                def gate_pair(g):
                    gi = pp_g.tile([DIM, BC], F32, tag="gpsum")
                    nc.tensor.matmul(
                        gi,
                        lhsT=WihT[hop][:, g * DIM : (g + 1) * DIM],
                        rhs=oT,
                        start=True,
                        stop=True,
                    )
                    gh = pp_g.tile([DIM, BC], F32, tag="gpsum")
                    nc.tensor.matmul(
                        gh,
                        lhsT=WhhT[hop][:, g * DIM : (g + 1) * DIM],
                        rhs=MT,
                        start=True,
                        stop=True,
                    )
                    return gi, gh

                # r,z gates: sigmoid(gi + gh + b_ih + b_hh)
                rz_t = []
                for g in range(2):
                    gi, gh = gate_pair(g)
                    gb = small.tile([DIM, BC], F32, tag=f"g{g}b")
                    nc.scalar.activation(gb, gi, AF.Identity, bias=bsum_rz[hop][g])
                    nc.vector.tensor_add(gb, gb, gh)
                    gt = small.tile([DIM, BC], F32, tag=f"gate{g}")
                    nc.scalar.activation(gt, gb, AF.Sigmoid)
                    rz_t.append(gt)
                r_t, z_t = rz_t

                # n = tanh(gi_n + b_ih_n + r * (gh_n + b_hh_n))
                gi_n, gh_n = gate_pair(2)
                ghn = small.tile([DIM, BC], F32, tag="ghn")
                nc.scalar.activation(ghn, gh_n, AF.Identity, bias=bhhn_t[hop])
                gin = small.tile([DIM, BC], F32, tag="gin")
                nc.scalar.activation(gin, gi_n, AF.Identity, bias=bihn_t[hop])
                n1 = small.tile([DIM, BC], F32, tag="n1")
                nc.vector.tensor_mul(n1, r_t, ghn)
                nc.vector.tensor_add(n1, n1, gin)
                n_t = small.tile([DIM, BC], F32, tag="n_t")
                nc.scalar.activation(n_t, n1, AF.Tanh)

                # M' = n + z * (M - n)
                MT_new = mstate.tile([DIM, BC], F32, tag="MT")
                nc.vector.tensor_sub(MT_new, MT, n_t)
                nc.vector.tensor_mul(MT_new, MT_new, z_t)
                nc.vector.tensor_add(MT_new, MT_new, n_t)
                MT = MT_new

                if hop < N_HOP - 1:
                    # rebuild M_rep (bf16) via DRAM bounce
                    mrow_ps = pp_g.tile([BC, DIM], F32, tag="gpsum")
                    nc.tensor.transpose(mrow_ps, MT, ident[:DIM, :DIM])
                    M16 = mstate.tile([BC, DIM], BF16, tag="M16")
                    nc.scalar.copy(out=M16, in_=mrow_ps)
                    nc.scalar.dma_start(out=m_flat[:, :], in_=M16)
                    nc.sync.dma_start(
                        out=m_scr2[:, :, :, :],
                        in_=m_flat.rearrange(
                            "(blk bp g) d -> bp blk g d", bp=2, g=NG
                        ),
                    )
                    M_rep_new = mstate.tile(
                        [128, NBLK * NG * DIM], BF16, tag="M_rep", bufs=1
                    )
                    nc.sync.dma_start(
                        out=m_scr3.rearrange("(m bp) f -> m bp f", bp=2),
                        in_=m_scr2.rearrange(
                            "bp blk g d -> bp (blk g d)"
                        ).partition_broadcast(64),
                    )
                    prev_bcast_dma = nc.sync.dma_start(
                        out=M_rep_new, in_=m_scr3[:, :]
                    )
                    M_rep = M_rep_new
                else:
                    # row-major M only needed for the final output
                    mrow_ps = pp_g.tile([BC, DIM], F32, tag="gpsum")
                    nc.tensor.transpose(mrow_ps, MT, ident[:DIM, :DIM])
                    M_row = mstate.tile([BC, DIM], F32, tag="M_row")
                    nc.scalar.copy(out=M_row, in_=mrow_ps)
                    nc.sync.dma_start(out=out_d[:, :], in_=M_row)

    nc.compile()
    return nc


_NC_CACHE = None


def _get_nc():
    global _NC_CACHE
    if _NC_CACHE is None:
        _NC_CACHE = build_nc()
    return _NC_CACHE


def _bf16(x):
    import ml_dtypes

    return np.asarray(x).astype(ml_dtypes.bfloat16)


def permute_local(x):
    """[BC, N_HOP, m, ...] -> [N_HOP, NBLK, m, 2, NG, ...] with b = blk*8+bp*4+g."""
    tail = x.shape[2:]
    y = x.reshape(NBLK, 2, NG, N_HOP, *tail)
    order = (3, 0, 4, 1, 2) + tuple(range(5, y.ndim))
    return np.ascontiguousarray(y.transpose(order))


def permute_h(x):
    """hs [BC, N_HOP, m, e] -> [N_HOP, m, 2, NBLK, NG, e]."""
    y = x.reshape(NBLK, 2, NG, N_HOP, N_MEM, DIM)
    return np.ascontiguousarray(y.transpose(3, 4, 1, 0, 2, 5))


def make_in_maps(hs, Rs, ts, vs, W1, b1, W2, W_ih, W_hh, b_ih, b_hh):
    in_maps = []
    for c in range(N_CORES):
        sl = slice(c * BC, (c + 1) * BC)
        in_maps.append(
            {
                "Rs": permute_local(_bf16(Rs[sl])),
                "hs": permute_h(_bf16(hs[sl])),
                "ts": _bf16(ts[sl]),
                "vs": np.ascontiguousarray(vs[sl]),
                "W1": np.ascontiguousarray(W1),
                "b1": np.ascontiguousarray(b1),
                "W2": np.ascontiguousarray(W2),
                "W_ih": np.ascontiguousarray(W_ih),
                "W_hh": np.ascontiguousarray(W_hh),
                "b_ih": np.ascontiguousarray(b_ih),
                "b_hh": np.ascontiguousarray(b_hh),
            }
        )
    return in_maps


def kernel(hs, Rs, ts, vs, W1, b1, W2, b2, W_ih, W_hh, b_ih, b_hh):
    from concourse.bass_utils import run_bass_kernel_spmd

    nc = _get_nc()
    in_maps = make_in_maps(hs, Rs, ts, vs, W1, b1, W2, W_ih, W_hh, b_ih, b_hh)
    res = run_bass_kernel_spmd(nc, in_maps, list(range(N_CORES)))
    return np.concatenate([r["out"] for r in res.results], axis=0)
